# revision 1
# baseline (speedup 1.0000x reference)
# HGNNP hypergraph convolution on 8 Trainium2 NeuronCores (Bass/Tile).
#
# Reference computation:
#   H      = relu(X @ W.T + b)                    [N, 128]
#   e_feat = segment_mean(H[v_idx], e_idx, E)     [E, 128]
#   out    = relu(segment_mean(e_feat[e_idx], v_idx, N))
#
# Strategy (vertex sharding, one kernel launch, in-kernel AllReduce):
#   * Each core owns a contiguous vertex range (VPC rows of X) and computes
#     its H shard with TensorE (X^T is pre-transposed on the host so the
#     contraction dim lands on partitions).
#   * Incidence entries are routed to the core owning their vertex.  Within a
#     core they are bucketed by edge-block (128 edges) and padded to a fixed
#     number of 128-entry tiles per block.  A dma_gather pulls H rows for each
#     entry; a per-tile 0/1 selection matrix (is_equal vs an iota row) and a
#     PSUM-accumulated matmul reduce the tile into the block's 128 edge rows.
#   * Per-core partial edge sums are AllReduced, scaled by 1/edge_degree, cast
#     to fp16 -> e_feat table.
#   * Phase B mirrors phase A keyed by vertex: gather e_feat rows per entry,
#     selection-matmul into 128-vertex blocks, scale by 1/vertex_degree, relu,
#     write the core's output rows.
#   * Gather payloads are fp16 (halves the dominant memory traffic);
#     accumulation stays fp32 in PSUM.
import os
from dataclasses import dataclass

import numpy as np

P = 128


def chunk_of(n):
    """Largest divisor of n that is a multiple of 128 and <= 1024 idxs
    (<= 64 descriptors per SDMA engine keeps single_packet legal)."""
    for ck in range(768, 127, -128):
        if n % ck == 0:
            return ck
    return 128


@dataclass(frozen=True)
class Cfg:
    n_cores: int
    in_ch: int
    out_ch: int
    vpc: int        # vertices per core (multiple of 128)
    ne_pad: int     # padded edge count (multiple of 128)
    ta: int         # 128-entry tiles per (core, edge-block)
    tv: int         # 128-entry tiles per (core, vertex-block)
    gb_a: int       # edge-blocks per gather in phase A
    gb_b: int       # vertex-blocks per gather in phase B

    @property
    def eb(self):
        return self.ne_pad // P

    @property
    def vb(self):
        return self.vpc // P

    @property
    def na(self):
        return self.eb * self.ta * P

    @property
    def nb(self):
        return self.vb * self.tv * P


# Real problem dimensions.
N_VERTICES = 100000
N_EDGES = 25000
NNZ = 3200000
NV_PAD = 100352           # 8 * 12544
REAL = dict(n_cores=8, in_ch=256, out_ch=128, vpc=12544, ne_pad=25088,
            gb_a=4, gb_b=2)

_PROG_CACHE = {}
LAST_RESULTS = None       # BassKernelResults of the most recent run (for test.py)


def build_program(cfg: Cfg):
    """Emit the SPMD Bass program (identical on all cores; per-core behavior
    comes entirely from per-core input tensors)."""
    import concourse.bass as bass
    import concourse.mybir as mybir
    import concourse.tile as tile
    from concourse import bacc

    dt = mybir.dt
    OC = cfg.out_ch
    assert cfg.in_ch % P == 0
    KC = cfg.in_ch // P

    nc = bacc.Bacc("TRN2", target_bir_lowering=False, debug=False,
                   num_devices=cfg.n_cores)

    # ---- I/O ----
    xt = nc.dram_tensor("xt", [cfg.in_ch, cfg.vpc], dt.float16, kind="ExternalInput")
    wt = nc.dram_tensor("wt", [cfg.in_ch, OC], dt.float16, kind="ExternalInput")
    bmat = nc.dram_tensor("bmat", [P, OC], dt.float32, kind="ExternalInput")
    iota = nc.dram_tensor("iota", [P, P], dt.float16, kind="ExternalInput")
    idxa = nc.dram_tensor("idxa", [P, cfg.na // 16], dt.int16, kind="ExternalInput")
    eloc = nc.dram_tensor("eloc", [P, cfg.na // P], dt.float16, kind="ExternalInput")
    idxb = nc.dram_tensor("idxb", [P, cfg.nb // 16], dt.int16, kind="ExternalInput")
    vloc = nc.dram_tensor("vloc", [P, cfg.nb // P], dt.float16, kind="ExternalInput")
    re_p = nc.dram_tensor("re", [P, cfg.eb], dt.float32, kind="ExternalInput")
    rv_p = nc.dram_tensor("rv", [P, cfg.vb], dt.float32, kind="ExternalInput")
    out = nc.dram_tensor("out", [cfg.vpc, OC], dt.float32, kind="ExternalOutput")

    # ---- internal DRAM ----
    hdr = nc.dram_tensor("hdram", [cfg.vpc + P, OC], dt.float16)
    esum = nc.dram_tensor("esum", [cfg.ne_pad, OC], dt.float16)
    esum_red = nc.dram_tensor("esum_red", [cfg.ne_pad, OC], dt.float16,
                              addr_space="Shared")
    efeat = nc.dram_tensor("efeat", [cfg.ne_pad + P, OC], dt.float16)

    def bcast_free(ap2d, n):
        # [P, C] -> [P, C, n] with the trailing dim broadcast
        return bass.AP(tensor=ap2d.tensor, offset=ap2d.offset,
                       ap=[*ap2d.ap, [0, n]])

    def bcast_mid(ap2d, n):
        # [P, C] -> [P, n, C] with the middle dim broadcast
        return bass.AP(tensor=ap2d.tensor, offset=ap2d.offset,
                       ap=[ap2d.ap[0], [0, n], ap2d.ap[1]])

    with tile.TileContext(nc) as tc:
        import contextlib
        with contextlib.ExitStack() as ctx:
            const = ctx.enter_context(tc.tile_pool(name="const", bufs=1))
            work = ctx.enter_context(tc.tile_pool(name="work", bufs=3))
            gpool = ctx.enter_context(tc.tile_pool(name="gpool", bufs=2))
            ipool = ctx.enter_context(tc.tile_pool(name="ipool", bufs=2))
            spool = ctx.enter_context(tc.tile_pool(name="spool", bufs=2))
            psum = ctx.enter_context(tc.tile_pool(name="psum", bufs=4, space="PSUM"))

            # ---- constants ----
            xt_sb = const.tile([P, KC, cfg.vpc], dt.float16)
            for k in range(KC):
                nc.sync.dma_start(out=xt_sb[:, k, :], in_=xt[k * P:(k + 1) * P, :])
            wt_sb = const.tile([P, KC, OC], dt.float16)
            for k in range(KC):
                nc.sync.dma_start(out=wt_sb[:, k, :], in_=wt[k * P:(k + 1) * P, :])
            bb = const.tile([P, OC], dt.float32)
            nc.sync.dma_start(out=bb[:], in_=bmat[:, :])
            iota_sb = const.tile([P, P], dt.float16)
            nc.sync.dma_start(out=iota_sb[:], in_=iota[:, :])
            eloc_sb = const.tile([P, cfg.na // P], dt.float16)
            nc.sync.dma_start(out=eloc_sb[:], in_=eloc[:, :])
            vloc_sb = const.tile([P, cfg.nb // P], dt.float16)
            nc.sync.dma_start(out=vloc_sb[:], in_=vloc[:, :])
            re_sb = const.tile([P, cfg.eb], dt.float32)
            nc.sync.dma_start(out=re_sb[:], in_=re_p[:, :])
            rv_sb = const.tile([P, cfg.vb], dt.float32)
            nc.sync.dma_start(out=rv_sb[:], in_=rv_p[:, :])

            # ---- stage H: H = relu(X @ W.T + b) -> fp16 rows in DRAM ----
            for vt in range(cfg.vb):
                ps = psum.tile([P, OC], dt.float32, space="PSUM")
                for k in range(KC):
                    nc.tensor.matmul(out=ps[:],
                                     lhsT=xt_sb[:, k, vt * P:(vt + 1) * P],
                                     rhs=wt_sb[:, k, :],
                                     start=(k == 0), stop=(k == KC - 1))
                tmp = work.tile([P, OC], dt.float32)
                nc.vector.tensor_add(out=tmp[:], in0=ps[:], in1=bb[:])
                h_t = work.tile([P, OC], dt.float16)
                nc.vector.tensor_scalar_max(out=h_t[:], in0=tmp[:], scalar1=0.0)
                nc.sync.dma_start(out=hdr[vt * P:(vt + 1) * P, :], in_=h_t[:])
            zt = work.tile([P, OC], dt.float16)
            nc.vector.memset(zt[:], 0.0)
            nc.sync.dma_start(out=hdr[cfg.vpc:cfg.vpc + P, :], in_=zt[:])

            # ---- phase A: partial edge sums ----
            n_ga = cfg.gb_a * cfg.ta * P          # idxs per gather
            assert cfg.eb % cfg.gb_a == 0
            for g in range(cfg.eb // cfg.gb_a):
                ixt = ipool.tile([P, n_ga // 16], dt.int16)
                nc.sync.dma_start(out=ixt[:],
                                  in_=idxa[:, g * (n_ga // 16):(g + 1) * (n_ga // 16)])
                gt = gpool.tile([P, cfg.gb_a * cfg.ta, OC], dt.float16)
                ck = chunk_of(n_ga)
                for q in range(n_ga // ck):
                    nc.gpsimd.dma_gather(
                        gt[:, q * (ck // P):(q + 1) * (ck // P), :], hdr[:, :],
                        ixt[:, q * (ck // 16):(q + 1) * (ck // 16)],
                        ck, ck, OC, single_packet=True)
                for j in range(cfg.gb_a):
                    ebi = g * cfg.gb_a + j
                    s_t = spool.tile([P, cfg.ta, P], dt.float16)
                    nc.vector.tensor_tensor(
                        out=s_t[:],
                        in0=bcast_free(eloc_sb[:, ebi * cfg.ta:(ebi + 1) * cfg.ta], P),
                        in1=bcast_mid(iota_sb[:, :], cfg.ta),
                        op=mybir.AluOpType.is_equal)
                    ps = psum.tile([P, OC], dt.float32, space="PSUM")
                    for t in range(cfg.ta):
                        nc.tensor.matmul(out=ps[:], lhsT=s_t[:, t, :],
                                         rhs=gt[:, j * cfg.ta + t, :],
                                         start=(t == 0), stop=(t == cfg.ta - 1))
                    es = work.tile([P, OC], dt.float16)
                    nc.vector.tensor_copy(out=es[:], in_=ps[:])
                    nc.sync.dma_start(out=esum[ebi * P:(ebi + 1) * P, :], in_=es[:])

            # ---- AllReduce partial edge sums ----
            nc.gpsimd.collective_compute(
                "AllReduce", mybir.AluOpType.add,
                replica_groups=[list(range(cfg.n_cores))],
                ins=[esum.ap().opt()], outs=[esum_red.ap().opt()])

            # ---- e_feat = esum_red * (1/edge_deg) -> fp16 table ----
            for et in range(cfg.eb):
                t_in = work.tile([P, OC], dt.float16)
                nc.sync.dma_start(out=t_in[:], in_=esum_red[et * P:(et + 1) * P, :])
                ef = work.tile([P, OC], dt.float16)
                nc.vector.tensor_scalar_mul(out=ef[:], in0=t_in[:],
                                            scalar1=re_sb[:, et:et + 1])
                nc.sync.dma_start(out=efeat[et * P:(et + 1) * P, :], in_=ef[:])
            ztb = work.tile([P, OC], dt.float16)
            nc.vector.memset(ztb[:], 0.0)
            nc.sync.dma_start(out=efeat[cfg.ne_pad:cfg.ne_pad + P, :], in_=ztb[:])

            # ---- phase B: vertex means + relu ----
            n_gb = cfg.gb_b * cfg.tv * P
            assert cfg.vb % cfg.gb_b == 0
            for g in range(cfg.vb // cfg.gb_b):
                ixt = ipool.tile([P, n_gb // 16], dt.int16)
                nc.sync.dma_start(out=ixt[:],
                                  in_=idxb[:, g * (n_gb // 16):(g + 1) * (n_gb // 16)])
                gt = gpool.tile([P, cfg.gb_b * cfg.tv, OC], dt.float16)
                ck = chunk_of(n_gb)
                for q in range(n_gb // ck):
                    nc.gpsimd.dma_gather(
                        gt[:, q * (ck // P):(q + 1) * (ck // P), :], efeat[:, :],
                        ixt[:, q * (ck // 16):(q + 1) * (ck // 16)],
                        ck, ck, OC, single_packet=True)
                for j in range(cfg.gb_b):
                    vbi = g * cfg.gb_b + j
                    s_t = spool.tile([P, cfg.tv, P], dt.float16)
                    nc.vector.tensor_tensor(
                        out=s_t[:],
                        in0=bcast_free(vloc_sb[:, vbi * cfg.tv:(vbi + 1) * cfg.tv], P),
                        in1=bcast_mid(iota_sb[:, :], cfg.tv),
                        op=mybir.AluOpType.is_equal)
                    ps = psum.tile([P, OC], dt.float32, space="PSUM")
                    for t in range(cfg.tv):
                        nc.tensor.matmul(out=ps[:], lhsT=s_t[:, t, :],
                                         rhs=gt[:, j * cfg.tv + t, :],
                                         start=(t == 0), stop=(t == cfg.tv - 1))
                    ot = work.tile([P, OC], dt.float32)
                    nc.vector.tensor_scalar(out=ot[:], in0=ps[:],
                                            scalar1=rv_sb[:, vbi:vbi + 1],
                                            scalar2=0.0,
                                            op0=mybir.AluOpType.mult,
                                            op1=mybir.AluOpType.max)
                    nc.sync.dma_start(out=out[vbi * P:(vbi + 1) * P, :], in_=ot[:])

    nc.compile()
    return nc


def pack_inputs(cfg: Cfg, X, W, b, v_idx, e_idx):
    """Host-side preprocessing: shard by vertex range, bucket entries, pad,
    and build the per-core input dicts."""
    f16, f32, i16 = np.float16, np.float32, np.int16
    C, VPC, EB, VB, TA, TV = cfg.n_cores, cfg.vpc, cfg.eb, cfg.vb, cfg.ta, cfg.tv
    NA, NB = cfg.na, cfg.nb
    nv_pad = C * VPC
    n_edges = int(e_idx.max()) + 1 if len(e_idx) else 0

    v = np.asarray(v_idx).astype(np.int64)
    e = np.asarray(e_idx).astype(np.int64)
    core = v // VPC

    # ----- phase A routing: bucket by (core, edge-block), any order inside -----
    blk = core * EB + e // P
    order = np.argsort(blk, kind="stable")
    cnt = np.bincount(blk, minlength=C * EB)
    assert cnt.max() <= TA * P, f"phase A padding overflow: {cnt.max()} > {TA * P}"
    starts = np.zeros(C * EB, np.int64)
    np.cumsum(cnt[:-1], out=starts[1:])
    ofs = np.arange(len(v), dtype=np.int64) - np.repeat(starts, cnt)
    blk_s = blk[order]
    core_s = blk_s // EB
    dest = core_s * NA + (blk_s % EB) * (TA * P) + ofs
    idxa_all = np.full(C * NA, VPC, i16)
    idxa_all[dest] = (v[order] - core_s * VPC).astype(i16)
    eloc_all = np.zeros(C * NA, f16)
    eloc_all[dest] = (e[order] % P).astype(f16)

    # ----- phase B routing: bucket by vertex-block -----
    blkb = v // P                      # == core * VB + local block
    order_b = np.argsort(blkb, kind="stable")
    cntb = np.bincount(blkb, minlength=C * VB)
    assert cntb.max() <= TV * P, f"phase B padding overflow: {cntb.max()} > {TV * P}"
    starts_b = np.zeros(C * VB, np.int64)
    np.cumsum(cntb[:-1], out=starts_b[1:])
    ofs_b = np.arange(len(v), dtype=np.int64) - np.repeat(starts_b, cntb)
    blkb_s = blkb[order_b]
    core_b = blkb_s // VB
    dest_b = core_b * NB + (blkb_s % VB) * (TV * P) + ofs_b
    idxb_all = np.full(C * NB, cfg.ne_pad, i16)
    idxb_all[dest_b] = e[order_b].astype(i16)
    vloc_all = np.zeros(C * NB, f16)
    vloc_all[dest_b] = (v[order_b] % P).astype(f16)

    # ----- degrees -----
    edeg = np.bincount(e, minlength=cfg.ne_pad).astype(f32)
    re = (1.0 / np.maximum(edeg, 1.0)).astype(f32)
    re_p = np.ascontiguousarray(re.reshape(EB, P).T)
    vdeg = np.bincount(v, minlength=nv_pad).astype(f32)
    rv = (1.0 / np.maximum(vdeg, 1.0)).astype(f32)

    # ----- dense inputs -----
    nv = X.shape[0]
    xt_full = np.zeros((cfg.in_ch, nv_pad), f16)
    xt_full[:, :nv] = np.asarray(X, np.float32).T.astype(f16)
    wt = np.ascontiguousarray(np.asarray(W, np.float32).T.astype(f16))
    bmat = np.tile(np.asarray(b, f32)[None, :], (P, 1))
    iota = np.tile(np.arange(P, dtype=f16)[None, :], (P, 1))

    def wrap16(a):
        # gather index layout: idx i -> [16 partitions, i // 16], replicated x8
        return np.ascontiguousarray(np.tile(a.reshape(-1, 16).T, (P // 16, 1)))

    def pack128(a):
        # per-tile column layout: entry i -> [i % 128, i // 128]
        return np.ascontiguousarray(a.reshape(-1, P).T)

    in_maps = []
    for c in range(C):
        in_maps.append({
            "xt": np.ascontiguousarray(xt_full[:, c * VPC:(c + 1) * VPC]),
            "wt": wt,
            "bmat": bmat,
            "iota": iota,
            "idxa": wrap16(idxa_all[c * NA:(c + 1) * NA]),
            "eloc": pack128(eloc_all[c * NA:(c + 1) * NA]),
            "idxb": wrap16(idxb_all[c * NB:(c + 1) * NB]),
            "vloc": pack128(vloc_all[c * NB:(c + 1) * NB]),
            "re": re_p,
            "rv": np.ascontiguousarray(rv[c * VPC:(c + 1) * VPC].reshape(VB, P).T),
        })
    return in_maps


def make_cfg(v_idx, e_idx, base=REAL):
    """Padding tile counts depend on the data; compute them here so the same
    builder serves any input of the real shapes."""
    v = np.asarray(v_idx).astype(np.int64)
    e = np.asarray(e_idx).astype(np.int64)
    eb = base["ne_pad"] // P
    vb = base["vpc"] // P
    blk = (v // base["vpc"]) * eb + e // P
    ta = int(np.ceil(np.bincount(blk, minlength=base["n_cores"] * eb).max() / P))
    blkb = v // P
    tv = int(np.ceil(np.bincount(blkb, minlength=base["n_cores"] * vb).max() / P))
    return Cfg(ta=max(ta, 1), tv=max(tv, 1), **base)


def run(cfg: Cfg, in_maps, trace=False):
    global LAST_RESULTS
    from concourse.bass_utils import run_bass_kernel_spmd
    key = (cfg.ta, cfg.tv)
    if key not in _PROG_CACHE:
        _PROG_CACHE[key] = build_program(cfg)
    nc = _PROG_CACHE[key]
    res = run_bass_kernel_spmd(nc, in_maps, core_ids=list(range(cfg.n_cores)),
                               trace=trace)
    LAST_RESULTS = res
    return res


def kernel(X, W, b, v_idx, e_idx, trace=False):
    cfg = make_cfg(v_idx, e_idx)
    in_maps = pack_inputs(cfg, X, W, b, v_idx, e_idx)
    res = run(cfg, in_maps, trace=trace)
    out = np.concatenate([res.results[c]["out"] for c in range(cfg.n_cores)], axis=0)
    return np.ascontiguousarray(out[:N_VERTICES]).astype(np.float32)



# revision 3
# speedup vs baseline: 1.0375x; 1.0375x over previous
# HGNNP hypergraph convolution on 8 Trainium2 NeuronCores (Bass/Tile).
#
# Reference computation:
#   H      = relu(X @ W.T + b)                    [N, 128]
#   e_feat = segment_mean(H[v_idx], e_idx, E)     [E, 128]
#   out    = relu(segment_mean(e_feat[e_idx], v_idx, N))
#
# Strategy: DENSE block-matmul formulation (zero gather descriptors).
#   On this part, indexed-DMA (dma_gather / dma_scatter_add) is descriptor-
#   rate-bound at ~8 ns per 256B descriptor regardless of payload or memory
#   (measured), so any per-entry gather design costs >= 2*NNZ/8 * 8ns ~ 6.4 ms
#   per core.  Instead we materialize the incidence matrix A (0/1 counts) as
#   fp8 tiles on the host and stream it from HBM at bulk rate (~242 GB/s):
#     phase A:  esum_cm[c, e]  = sum_vb  H_blk[vb]^T     @ A1[vb, e-chunk]
#     phase B:  out_cm[c, v]   = sum_eb  efeat_blk[eb]^T @ A2[eb, v-chunk]
#   Both phases contract on the TensorEngine with the small dense operand
#   (H block / e_feat block, fp16) stationary and fp8 incidence chunks
#   streaming.  fp8 e4m3 holds small integer counts exactly, so the
#   segment sums are exact; per-core partial edge sums are AllReduced in
#   fp16 and scaled by 1/deg.  ~315 MB of A per phase per core at bulk HBM
#   rate ~= 1.3 ms/phase, matching the memory roofline for this regime.
import numpy as np

P = 128

N_VERTICES = 100000
N_EDGES = 25000
IN_CH = 256
OUT_CH = 128
N_CORES = 8

VPC = 12544              # vertices per core (98 blocks of 128)
VB = VPC // P            # 98
NV_PAD = N_CORES * VPC   # 100352
NE_PAD = 25088           # 196 blocks of 128
EB = NE_PAD // P         # 196

# phase A: 49 e-chunks of 512, in 7 passes x 7 chunks (7 PSUM banks)
ECW = 512
EPASS, ECHK = 7, 7       # 7*7*512 == 25088
# phase B: 25 v-chunks of 512 on 12800 padded rows, 5 passes x 5 chunks
VCW = 512
VPASS, VCHK = 5, 5       # 5*5*512 == 12800
VPC_B = VPASS * VCHK * VCW   # 12800
VBB = VPC_B // P         # 100 output blocks

_PROG_CACHE = {}
LAST_RESULTS = None      # BassKernelResults of the most recent run (for test.py)


def build_program():
    import concourse.mybir as mybir
    import concourse.tile as tile
    from concourse import bacc

    dt = mybir.dt
    KC = IN_CH // P      # 2

    nc = bacc.Bacc("TRN2", target_bir_lowering=False, debug=False,
                   num_devices=N_CORES)

    # ---- I/O ----
    xt = nc.dram_tensor("xt", [IN_CH, VPC], dt.float16, kind="ExternalInput")
    wt = nc.dram_tensor("wt", [IN_CH, OUT_CH], dt.float16, kind="ExternalInput")
    bmat = nc.dram_tensor("bmat", [P, OUT_CH], dt.float32, kind="ExternalInput")
    ident = nc.dram_tensor("ident", [P, P], dt.float16, kind="ExternalInput")
    a1 = nc.dram_tensor("a1", [EPASS * VB * P, ECHK * ECW], dt.float8e4,
                        kind="ExternalInput")
    a2 = nc.dram_tensor("a2", [VPASS * EB * P, VCHK * VCW], dt.float8e4,
                        kind="ExternalInput")
    re_p = nc.dram_tensor("re", [P, EB], dt.float32, kind="ExternalInput")
    rv_p = nc.dram_tensor("rv", [P, VBB], dt.float32, kind="ExternalInput")
    out = nc.dram_tensor("out", [VPC_B, OUT_CH], dt.float32,
                         kind="ExternalOutput")

    # ---- internal DRAM ----
    esum = nc.dram_tensor("esum", [P, NE_PAD], dt.float16)          # ch-major
    esum_red = nc.dram_tensor("esum_red", [P, NE_PAD], dt.float16,
                              addr_space="Shared")

    with tile.TileContext(nc) as tc:
        import contextlib
        with contextlib.ExitStack() as ctx:
            const = ctx.enter_context(tc.tile_pool(name="const", bufs=1))
            hpool = ctx.enter_context(tc.tile_pool(name="hpool", bufs=1))
            efpool = ctx.enter_context(tc.tile_pool(name="efpool", bufs=1))
            apool = ctx.enter_context(tc.tile_pool(name="apool", bufs=3))
            work = ctx.enter_context(tc.tile_pool(name="work", bufs=3))
            # 7 rotating PSUM slot names (one bank each); all stages share
            psA = ctx.enter_context(tc.tile_pool(name="psA", bufs=1,
                                                 space="PSUM"))

            # ---- constants ----
            xt_sb = const.tile([P, KC, VPC], dt.float16)
            for k in range(KC):
                nc.sync.dma_start(out=xt_sb[:, k, :], in_=xt[k * P:(k + 1) * P, :])
            wt_sb = const.tile([P, KC, OUT_CH], dt.float16)
            for k in range(KC):
                nc.sync.dma_start(out=wt_sb[:, k, :], in_=wt[k * P:(k + 1) * P, :])
            bb = const.tile([P, OUT_CH], dt.float32)
            nc.sync.dma_start(out=bb[:], in_=bmat[:, :])
            id_sb = const.tile([P, P], dt.float16)
            nc.sync.dma_start(out=id_sb[:], in_=ident[:, :])
            re_sb = const.tile([P, EB], dt.float32)
            nc.sync.dma_start(out=re_sb[:], in_=re_p[:, :])
            rv_sb = const.tile([P, VBB], dt.float32)
            nc.sync.dma_start(out=rv_sb[:], in_=rv_p[:, :])

            # ---- stage H: H = relu(X @ W.T + b), fp16 blocks in SBUF ----
            # h_sb[vr, vb, c] = H[vb*128+vr, c]
            h_sb = hpool.tile([P, VB, OUT_CH], dt.float16)
            for vb in range(VB):
                ps = psA.tile([P, OUT_CH], dt.float32, space="PSUM",
                              name=f"ps{vb % 2}")
                for k in range(KC):
                    nc.tensor.matmul(out=ps[:],
                                     lhsT=xt_sb[:, k, vb * P:(vb + 1) * P],
                                     rhs=wt_sb[:, k, :],
                                     start=(k == 0), stop=(k == KC - 1))
                tmp = work.tile([P, OUT_CH], dt.float32)
                nc.vector.tensor_add(out=tmp[:], in0=ps[:], in1=bb[:])
                nc.vector.tensor_scalar_max(out=h_sb[:, vb, :], in0=tmp[:],
                                            scalar1=0.0)

            # ---- phase A: esum_cm[c, e] = sum_vb H[vb]^T @ A1[vb, echunk] ----
            for sp in range(EPASS):
                pss = [psA.tile([P, ECW], dt.float32, space="PSUM",
                                name=f"ps{j}") for j in range(ECHK)]
                for vb in range(VB):
                    a1t = apool.tile([P, ECHK * ECW], dt.float8e4)
                    r0 = (sp * VB + vb) * P
                    nc.sync.dma_start(out=a1t[:], in_=a1[r0:r0 + P, :])
                    for j in range(ECHK):
                        nc.tensor.matmul(out=pss[j][:],
                                         lhsT=h_sb[:, vb, :],
                                         rhs=a1t[:, j * ECW:(j + 1) * ECW],
                                         start=(vb == 0), stop=(vb == VB - 1))
                for j in range(ECHK):
                    es = work.tile([P, ECW], dt.float16)
                    nc.vector.tensor_copy(out=es[:], in_=pss[j][:])
                    c0 = (sp * ECHK + j) * ECW
                    nc.sync.dma_start(out=esum[:, c0:c0 + ECW], in_=es[:])

            # ---- AllReduce partial edge sums (channel-major, fp16) ----
            nc.gpsimd.collective_compute(
                "AllReduce", mybir.AluOpType.add,
                replica_groups=[list(range(N_CORES))],
                ins=[esum.ap().opt()], outs=[esum_red.ap().opt()])

            # ---- e_feat blocks: transpose each eb block, scale by 1/e_deg ----
            # ef_sb[er, eb, c] = esum_red[c, eb*128+er] * re[er, eb]
            ef_sb = efpool.tile([P, EB, OUT_CH], dt.float16)
            EBG = 28                         # eb blocks per bulk load
            for g in range(EB // EBG):
                ech = apool.tile([P, EBG * P], dt.float16)
                nc.sync.dma_start(out=ech[:],
                                  in_=esum_red[:, g * EBG * P:(g + 1) * EBG * P])
                for s in range(EBG):
                    eb = g * EBG + s
                    pst = psA.tile([P, P], dt.float16, space="PSUM",
                                   name=f"ps{s % 2}")
                    nc.tensor.transpose(pst[:], ech[:, s * P:(s + 1) * P],
                                        id_sb[:])
                    nc.vector.tensor_scalar_mul(out=ef_sb[:, eb, :], in0=pst[:],
                                                scalar1=re_sb[:, eb:eb + 1])

            # ---- phase B: out_cm[c, v] = sum_eb ef[eb]^T @ A2[eb, vchunk] ----
            for sp in range(VPASS):
                pss = [psA.tile([P, VCW], dt.float32, space="PSUM",
                                name=f"ps{j}") for j in range(VCHK)]
                for eb in range(EB):
                    a2t = apool.tile([P, VCHK * VCW], dt.float8e4)
                    r0 = (sp * EB + eb) * P
                    nc.sync.dma_start(out=a2t[:], in_=a2[r0:r0 + P, :])
                    for j in range(VCHK):
                        nc.tensor.matmul(out=pss[j][:],
                                         lhsT=ef_sb[:, eb, :],
                                         rhs=a2t[:, j * VCW:(j + 1) * VCW],
                                         start=(eb == 0), stop=(eb == EB - 1))
                for j in range(VCHK):
                    cm = work.tile([P, VCW], dt.float16)
                    nc.vector.tensor_copy(out=cm[:], in_=pss[j][:])
                    for b in range(VCW // P):
                        vbb = (sp * VCHK + j) * (VCW // P) + b
                        pst = psA.tile([P, P], dt.float16, space="PSUM",
                                       name=f"ps{5 + b % 2}")
                        nc.tensor.transpose(pst[:], cm[:, b * P:(b + 1) * P],
                                            id_sb[:])
                        ot = work.tile([P, OUT_CH], dt.float32)
                        nc.vector.tensor_scalar(out=ot[:], in0=pst[:],
                                                scalar1=rv_sb[:, vbb:vbb + 1],
                                                scalar2=0.0,
                                                op0=mybir.AluOpType.mult,
                                                op1=mybir.AluOpType.max)
                        nc.sync.dma_start(out=out[vbb * P:(vbb + 1) * P, :],
                                          in_=ot[:])

    nc.compile()
    return nc


def pack_inputs(X, W, b, v_idx, e_idx):
    """Host-side preprocessing: build per-core fp8 incidence tiles in the
    pass/block-chunk layouts the device program streams, plus dense inputs."""
    import ml_dtypes
    f16, f32 = np.float16, np.float32
    f8 = ml_dtypes.float8_e4m3

    v = np.asarray(v_idx).astype(np.int64)
    e = np.asarray(e_idx).astype(np.int64)

    # fp8 byte LUT for small counts (0..15); counts beyond 15 are impossible
    # for random data but clip defensively (value error stays tiny/local).
    lut = np.arange(16, dtype=np.float32).astype(f8).view(np.uint8)

    # dense inputs
    xt_full = np.zeros((IN_CH, NV_PAD), f16)
    xt_full[:, :N_VERTICES] = np.asarray(X, f32).T.astype(f16)
    wt = np.ascontiguousarray(np.asarray(W, f32).T.astype(f16))
    bmat = np.tile(np.asarray(b, f32)[None, :], (P, 1))
    ident = np.eye(P, dtype=f16)

    # degree reciprocals
    edeg = np.bincount(e, minlength=NE_PAD).astype(f32)
    re = (1.0 / np.maximum(edeg, 1.0)).astype(f32)
    re_p = np.ascontiguousarray(re.reshape(EB, P).T)          # [er, eb]
    vdeg = np.bincount(v, minlength=N_CORES * VPC_B).astype(f32)
    rv = (1.0 / np.maximum(vdeg, 1.0)).astype(f32)

    core = v // VPC
    vl = v - core * VPC

    in_maps = []
    for c in range(N_CORES):
        m = core == c
        vc, ec = vl[m], e[m]

        # a1[(sp*VB+vb)*P + vr, ecp*ECW + el] = count(v==vb*P+vr,
        #                                             e==(sp*ECHK+ecp)*ECW+el)
        a1_u8 = np.zeros((EPASS, VB, P, ECHK, ECW), np.uint8)
        sp = ec // (ECHK * ECW)
        rem = ec - sp * (ECHK * ECW)
        ecp = rem // ECW
        el = rem - ecp * ECW
        np.add.at(a1_u8, (sp, vc // P, vc % P, ecp, el), 1)
        a1 = lut[np.minimum(a1_u8, 15)].view(f8).reshape(EPASS * VB * P,
                                                         ECHK * ECW)
        del a1_u8

        # a2[(sp*EB+eb)*P + er, vcp*VCW + vl] = count(e==eb*P+er,
        #                                             v==(sp*VCHK+vcp)*VCW+vl)
        a2_u8 = np.zeros((VPASS, EB, P, VCHK, VCW), np.uint8)
        spv = vc // (VCHK * VCW)
        remv = vc - spv * (VCHK * VCW)
        vcp = remv // VCW
        vlo = remv - vcp * VCW
        np.add.at(a2_u8, (spv, ec // P, ec % P, vcp, vlo), 1)
        a2 = lut[np.minimum(a2_u8, 15)].view(f8).reshape(VPASS * EB * P,
                                                         VCHK * VCW)
        del a2_u8

        rv_core = rv[c * VPC:(c + 1) * VPC]
        rv_pad = np.zeros(VPC_B, f32)
        rv_pad[:VPC] = rv_core
        in_maps.append({
            "xt": np.ascontiguousarray(xt_full[:, c * VPC:(c + 1) * VPC]),
            "wt": wt,
            "bmat": bmat,
            "ident": ident,
            "a1": a1,
            "a2": a2,
            "re": re_p,
            "rv": np.ascontiguousarray(rv_pad.reshape(VBB, P).T),
        })
    return in_maps


def run(in_maps, trace=False):
    global LAST_RESULTS
    from concourse.bass_utils import run_bass_kernel_spmd
    if "prog" not in _PROG_CACHE:
        _PROG_CACHE["prog"] = build_program()
    nc = _PROG_CACHE["prog"]
    res = run_bass_kernel_spmd(nc, in_maps, core_ids=list(range(N_CORES)),
                               trace=trace)
    LAST_RESULTS = res
    return res


def kernel(X, W, b, v_idx, e_idx, trace=False):
    in_maps = pack_inputs(X, W, b, v_idx, e_idx)
    res = run(in_maps, trace=trace)
    out = np.concatenate([res.results[c]["out"][:VPC] for c in range(N_CORES)],
                         axis=0)
    return np.ascontiguousarray(out[:N_VERTICES]).astype(np.float32)
